# revision 6
# baseline (speedup 1.0000x reference)
"""Depth-weighted 3x3 conv (DepthConv) Trainium2 Bass kernel (fp16).

Math (per batch element):
  sim[k, p] = exp(-|depth[p + off_k] - depth[p]|)   (9 taps, off = dh*W + dw)
  out[o, p] = sum_{c,k} W[o,c,k] * sim[k,p] * x[c, p + off_k] + bias[o]

Sharding: data-parallel over batch, one batch element per NeuronCore (8).

Layout (per core): SBUF partitions = 64 channels x {top, bottom} half.
Free dim = padded flat image, WB=164 per row ([P P x0..x159 P P]), 84
rows; all row starts EVEN so every DVE op is 4B-aligned (2x mode).  Out
pixel (j, w) center q = (j+2)*164 + 2 + w in both halves.  x2o is the
parity-shifted copy (x2o[i] = x2e[i+1]) built ON-CHIP by DVE copy so
odd-offset tap products keep the DVE 2x alignment.

Host prep (make_in_maps, pure layout transforms): x pre-padded into the
x2e layout (fp16); depth pre-padded per half in an overlapping 16-chunk
layout for SBUF residence; weights BLOCK-DIAGONAL wt2[64h+c, t, 64h'+o]
= W[o,c,t]*(h==h')/255 so one [128,128] lhsT drives both image halves
on the full PE array in a single matmul per tap (the /255 undoes the u8
sim scaling below).

Device pipeline (all math on device):
  - compact sim: two stride-paired DVE subs directly on the depth tile
    (map slots ordered (1,164,163,165) so sub pairs have uniform
    strides), ACT |x|, then ACT exp with scale=-1, bias=ln(255) writing
    UINT8: tsu8 = round(255*exp(-|dd|)) -- a [32, 4*861] u8 tile.
  - linearize to DRAM u8 (s8d), then per segment one SWDGE cast-DMA per
    half replicates 4 maps x64 partitions into sim_b [128, 4*1808] fp16
    (u8 source halves the HBM read traffic of the 64x broadcast).
  - products: ONE merged 4-map prod mul (stride-0 x-repeat) + xm muls
    (slots 2,3 merged via stride-2 x2o reads) -- all DVE 2x mode.
  - 9 taps accumulate into two [128,1024] fp32 PSUMs per segment, TAP-
    OUTER order (one weight load per tap instead of four); ACT
    evacuates with fused bias, stripping row pads via strided PSUM
    read; fp16 out DMA.
  - 8 segments fully pipelined (sim broadcast prefetched 2 ahead,
    x chunk B loaded during the loop).
"""

import functools
import math
import os
import sys

import numpy as np

for _p in ("/opt/trn_rl_repo",):
    if os.path.isdir(_p) and _p not in sys.path:
        sys.path.insert(0, _p)

import concourse.bass as bass
import concourse.mybir as mybir
import concourse.tile as tile
from concourse import bacc
from concourse.bass_utils import run_bass_kernel_spmd

# ---------------------------------------------------------------- constants
B, C, H, W = 8, 64, 160, 160
O = 64
KK = 9
WB = 164                   # padded row: [P P x0..x159 P P]
NROWG = 84                 # buffer rows per half
FLATG = NROWG * WB         # 13776
DPAD_W = FLATG + 2 * WB    # depth pad width (covers shifted reads)
CCH = FLATG // 16          # 861: compact-sim column chunk
Q0 = 2 * WB + 2            # center flat index of out pixel (0,0): 330
NCORES = 8

NSEG = 8
SEGROWS = 10               # out-rows per segment per half
SEGQ = SEGROWS * WB        # 1640
HALO = 166                 # even, >= max |off| (165)
PSPAN = SEGQ + HALO        # 1806: prod tile used span
WINB = 1808                # sim tile alloc stride (even)
CHW = 5 * WB               # 820 q-span per psum chunk
SUBS = (512, 308)          # matmul N splits at fp32 PSUM bank boundary
NCHUNK = 2                 # psum chunks per segment
XSPLIT = 44 * WB           # x load chunk boundary (segments 0-3 vs 4-7)

# (dh, dw, off) for the 4 positive-offset maps, in SLOT order chosen so
# the compact-sim subs pair with uniform strides: slots (0,1) offs (1,164)
# stride 163; slots (2,3) offs (163,165) stride 2.
MAPS = [(0, 1, 1), (1, 0, WB), (1, -1, WB - 1), (1, 1, WB + 1)]
DCHW = 1032                # depth chunk width in SBUF (CCH + halo, even)

LOG255 = float(math.log(255.0))

F32 = mybir.dt.float32
F16 = mybir.dt.float16
U8 = mybir.dt.uint8


def _tapidx(dh, dw):
    return (dh + 1) * 3 + (dw + 1)


def _build_program():
    nc = bacc.Bacc(None)
    x2e_d = nc.declare_dram_parameter("x2e", [128, FLATG], F16, isOutput=False)
    # depth in overlapping 16-chunk layout: dsb[16h+k, i] = D_h[k*CCH + i]
    dp_d = nc.declare_dram_parameter("dsb", [32, DCHW], F16, isOutput=False)
    # block-diagonal weights: wt2[64h+c, t, 64h'+o] = W[o,c,t]/255 * (h==h'),
    # so ONE [128,128] lhsT drives both halves on the full PE array
    wt_d = nc.declare_dram_parameter("wt2", [128, KK, 128], F16, isOutput=False)
    b_d = nc.declare_dram_parameter("bias2", [2 * O], F32, isOutput=False)
    out_d = nc.declare_dram_parameter("out", [O, H, W], F16, isOutput=True)

    Exp = mybir.ActivationFunctionType.Exp
    Abs = mybir.ActivationFunctionType.Abs
    Ident = mybir.ActivationFunctionType.Identity

    with tile.TileContext(nc) as tc:
        with (
            tc.tile_pool(name="dramp", bufs=1, space="DRAM") as dramp,
            tc.tile_pool(name="singles", bufs=1) as singles,
            tc.tile_pool(name="simp", bufs=3) as simp,
            tc.tile_pool(name="prodp", bufs=2) as prodp,
            tc.tile_pool(name="xmp", bufs=4) as xmp,
            tc.tile_pool(name="xm13p", bufs=2) as xm13p,
            tc.tile_pool(name="stgp", bufs=2) as stgp,
            tc.tile_pool(name="cpsum", bufs=4, space="PSUM") as cpsum,
        ):
            x2e = singles.tile([128, FLATG], F16)
            x2o = singles.tile([128, FLATG], F16)
            wt = singles.tile([128, KK, 128], F16)
            b2 = singles.tile([128, 1], F32)
            dsb = singles.tile([32, DCHW], F16)
            ts32 = singles.tile([32, 4 * CCH], F16)
            tsu8 = singles.tile([32, 4 * CCH], U8)
            ln255 = singles.tile([32, 1], F32)
            nc.vector.memset(ln255[:], LOG255)

            # ---------------- depth to SBUF once; the compact sim then
            # never touches HBM until the s8d linearize.  ts32[16h+k,
            # slot*CCH + i] = D_h[q+off_slot] - D_h[q], q = k*CCH+i, via
            # two stride-paired subs directly on dsb (no fill DMAs).
            nc.sync.dma_start(out=dsb[:], in_=dp_d[:])
            t32f = ts32[:]
            dsbf = dsb[:]

            def _sub_pair(slot0, in0_off, in0_stride):
                nc.vector.tensor_sub(
                    bass.AP(
                        tensor=t32f.tensor,
                        offset=t32f.offset + slot0 * CCH,
                        ap=[list(t32f.ap[0]), [CCH, 2], [1, CCH]],
                    ),
                    bass.AP(
                        tensor=dsbf.tensor,
                        offset=dsbf.offset + in0_off,
                        ap=[list(dsbf.ap[0]), [in0_stride, 2], [1, CCH]],
                    ),
                    bass.AP(
                        tensor=dsbf.tensor,
                        offset=dsbf.offset,
                        ap=[list(dsbf.ap[0]), [0, 2], [1, CCH]],
                    ),
                )

            _sub_pair(0, 1, WB - 1)      # slots 0,1: offs 1, 164
            _sub_pair(2, WB - 1, 2)      # slots 2,3: offs 163, 165
            nc.scalar.activation(out=ts32[:], in_=ts32[:], func=Abs)
            # u8 sim: round(255 * exp(-|dd|)) = Exp(-|dd| + ln 255)
            nc.scalar.activation(
                out=tsu8[:], in_=ts32[:], func=Exp, scale=-1.0, bias=ln255[:]
            )

            # linearize: s8d row r = 4h+slot, flat chunk k at k*CCH (u8)
            s8d = dramp.tile([8, FLATG], U8)
            s8d_f = s8d[:]
            for h in range(2):
                dst = bass.AP(
                    tensor=s8d_f.tensor,
                    offset=s8d_f.offset + 4 * h * FLATG,
                    ap=[[0, 1], [CCH, 16], [FLATG, 4], [1, CCH]],
                )
                eng = nc.sync if h == 0 else nc.scalar
                eng.dma_start(out=dst, in_=tsu8[16 * h : 16 * h + 16, :])

            # ---------------- x chunk A + weights.  Chunk B (needed from
            # segment 4) is deferred into the loop.  x2o = x2e shifted by
            # one element, built on-chip (saves a full 3.5 MB HBM load).
            nc.sync.dma_start(out=x2e[:, 0:XSPLIT], in_=x2e_d[:, 0:XSPLIT])
            nc.scalar.dma_start(out=wt[:], in_=wt_d[:])
            nc.scalar.dma_start(
                out=b2[:], in_=b_d.rearrange("(p one) -> p one", one=1)
            )
            nc.vector.tensor_copy(
                out=x2o[:, 0 : XSPLIT - 1], in_=x2e[:, 1:XSPLIT]
            )

            # ---------------- main loop
            def emit_bcast(s):
                """One cast-DMA per half: 4 u8 maps replicated x64 from
                DRAM, converted to fp16 (values 0..255) on the fly."""
                winbase = Q0 + s * SEGQ - HALO
                sim_b = simp.tile([128, 4 * WINB], F16, tag="sim")
                sbv = sim_b.rearrange("p (m i) -> p m i", m=4, i=WINB)
                for h in range(2):
                    src = bass.AP(
                        tensor=s8d_f.tensor,
                        offset=s8d_f.offset + 4 * h * FLATG + winbase,
                        ap=[[0, 64], [FLATG, 4], [1, WINB]],
                    )
                    nc.gpsimd.dma_start(out=sbv[64 * h : 64 * h + 64], in_=src)
                return sim_b

            sim_tiles = [emit_bcast(0), emit_bcast(1)]

            for s in range(NSEG):
                qs = Q0 + s * SEGQ
                winbase = qs - HALO

                if s == 1:
                    nc.scalar.dma_start(
                        out=x2e[:, XSPLIT:], in_=x2e_d[:, XSPLIT:]
                    )
                    nc.vector.tensor_copy(
                        out=x2o[:, XSPLIT - 1 : FLATG - 1],
                        in_=x2e[:, XSPLIT:FLATG],
                    )
                if s + 2 < NSEG:
                    sim_tiles.append(emit_bcast(s + 2))

                sim_b = sim_tiles[s]
                sbv = sim_b.rearrange("p (m i) -> p m i", m=4, i=WINB)

                # merged 4-map prod: in0 = x2e window repeated (stride 0)
                prod_b = prodp.tile([128, 4 * WINB], F16, tag="prod")
                pbv = prod_b.rearrange("p (m i) -> p m i", m=4, i=WINB)
                x2e_f = x2e[:]
                xrep = bass.AP(
                    tensor=x2e_f.tensor,
                    offset=x2e_f.offset + winbase,
                    ap=[list(x2e_f.ap[0]), [0, 4], [1, PSPAN]],
                )
                nc.vector.tensor_mul(
                    pbv[:, :, 0:PSPAN], xrep, sbv[:, :, 0:PSPAN]
                )

                # xm products: slots 0,1 single ops; slots 2,3 (offs
                # 163/165) merged via stride-2 x2o reads
                xm0 = xmp.tile([128, SEGQ], F16, tag="xm")
                nc.vector.tensor_mul(
                    xm0[:], x2o[:, qs : qs + SEGQ], sbv[:, 0, HALO : HALO + SEGQ]
                )
                xm1 = xmp.tile([128, SEGQ], F16, tag="xm")
                nc.vector.tensor_mul(
                    xm1[:],
                    x2e[:, qs + WB : qs + WB + SEGQ],
                    sbv[:, 1, HALO : HALO + SEGQ],
                )
                xm23 = xm13p.tile([128, 2 * SEGQ], F16, tag="xm23")
                x2o_f = x2o[:]
                sb_f = sim_b[:]
                nc.vector.tensor_mul(
                    bass.AP(
                        tensor=xm23[:].tensor,
                        offset=xm23[:].offset,
                        ap=[list(xm23[:].ap[0]), [SEGQ, 2], [1, SEGQ]],
                    ),
                    bass.AP(
                        tensor=x2o_f.tensor,
                        offset=x2o_f.offset + qs + WB - 2,
                        ap=[list(x2o_f.ap[0]), [2, 2], [1, SEGQ]],
                    ),
                    bass.AP(
                        tensor=sb_f.tensor,
                        offset=sb_f.offset + 2 * WINB + HALO,
                        ap=[list(sb_f.ap[0]), [WINB, 2], [1, SEGQ]],
                    ),
                )

                # tap sources: (weight idx, tile, base offset); actual rhs
                # window = base + j*CHW + o2.  Center tap first: it only
                # needs x2e, so the PE can open the psum groups before the
                # DVE products for this segment land.
                tapsrc = [(_tapidx(0, 0), x2e, qs)]
                for m, (dh, dw, off) in enumerate(MAPS):
                    tapsrc.append(
                        (_tapidx(-dh, -dw), prod_b, m * WINB + HALO - off)
                    )
                xms = [xm0, xm1, xm23, xm23]
                xmoff = [0, 0, 0, SEGQ]
                for m, (dh, dw, off) in enumerate(MAPS):
                    tapsrc.append((_tapidx(dh, dw), xms[m], xmoff[m]))

                # matmuls TAP-OUTER: one weight load per tap, 4 matmuls
                # (2 chunks x 2 bank-subs) with the same stationary lhsT.
                psums = []
                for _j in range(NCHUNK):
                    cps = cpsum.tile([128, 1024], F32, tag="cps")
                    psums.append(cps)
                ntap = len(tapsrc)
                for ti, (widx, rsrc, rbase) in enumerate(tapsrc):
                    for j in range(NCHUNK):
                        o2 = 0
                        for nn in SUBS:
                            roff = rbase + j * CHW + o2
                            nc.tensor.matmul(
                                psums[j][:, o2 : o2 + nn],
                                wt[:, widx, :],
                                rsrc[:, roff : roff + nn],
                                start=(ti == 0),
                                stop=(ti == ntap - 1),
                                skip_group_check=True,
                            )
                            o2 += nn

                # strip pad columns: psum rows of 164 -> 160
                stg = stgp.tile([128, SEGROWS * W], F16, tag="stg")
                for j in range(NCHUNK):
                    psum = psums[j]
                    nc.scalar.activation(
                        out=stg[:, j * 5 * W : (j + 1) * 5 * W].rearrange(
                            "p (r w) -> p r w", r=5, w=W
                        ),
                        in_=bass.AP(
                            tensor=psum[:].tensor,
                            offset=psum[:].offset,
                            ap=[list(psum[:].ap[0]), [WB, 5], [1, W]],
                        ),
                        func=Ident,
                        bias=b2[:],
                        scale=1.0,
                    )

                r0o = SEGROWS * s
                if s == NSEG - 1:
                    # final segment: flush per 5-row chunk so the last out
                    # DMA starts right after the last evacuation
                    for j in range(NCHUNK):
                        ra = r0o + 5 * j
                        sl = slice(j * 5 * W, (j + 1) * 5 * W)
                        nc.sync.dma_start(
                            out=out_d[:, ra : ra + 5, :].rearrange(
                                "c r w -> c (r w)"
                            ),
                            in_=stg[0:64, sl],
                        )
                        nc.sync.dma_start(
                            out=out_d[:, 80 + ra : 80 + ra + 5, :].rearrange(
                                "c r w -> c (r w)"
                            ),
                            in_=stg[64:128, sl],
                        )
                else:
                    nc.sync.dma_start(
                        out=out_d[:, r0o : r0o + SEGROWS, :].rearrange(
                            "c r w -> c (r w)"
                        ),
                        in_=stg[0:64, :],
                    )
                    nc.sync.dma_start(
                        out=out_d[
                            :, 80 + r0o : 80 + r0o + SEGROWS, :
                        ].rearrange("c r w -> c (r w)"),
                        in_=stg[64:128, :],
                    )

    return nc


@functools.lru_cache(maxsize=1)
def _get_program():
    return _build_program()


def make_in_maps(x, depth, weights, bias):
    x = np.asarray(x, np.float32)
    depth = np.asarray(depth, np.float32)
    # /255 undoes the u8 sim scaling -- except the center tap, whose rhs
    # is raw x (sim == 1 exactly, never multiplied by the 255-scaled sim)
    wscale = np.full((1, 1, KK), 1.0 / 255.0)
    wscale[0, 0, (KK // 2)] = 1.0
    wbase = np.ascontiguousarray(
        weights.reshape(O, C, KK) * wscale
    ).transpose(1, 2, 0).astype(np.float16)
    wt2 = np.zeros((128, KK, 128), np.float16)
    wt2[0:64, :, 0:64] = wbase
    wt2[64:128, :, 64:128] = wbase
    b2 = np.concatenate([bias, bias]).astype(np.float32)

    n = x.shape[0]
    # padded layouts (pure layout transforms; all math stays on device)
    x2e = np.zeros((n, 128, NROWG, WB), np.float16)
    x2e[:, 0:64, 2:83, 2:162] = x[:, :, 0:81, :]
    x2e[:, 64:128, 1:82, 2:162] = x[:, :, 79:160, :]
    x2e = x2e.reshape(n, 128, FLATG)

    dpad = np.zeros((n, 2, DPAD_W), np.float16)
    dpv = dpad.reshape(n, 2, DPAD_W // WB, WB)
    dpv[:, 0, 2:83, 2:162] = depth[:, 0, 0:81, :]
    dpv[:, 1, 1:82, 2:162] = depth[:, 0, 79:160, :]
    # overlapping 16-chunk layout for SBUF residence
    dsb = np.zeros((n, 32, DCHW), np.float16)
    for k in range(16):
        w = min(DCHW, DPAD_W - k * CCH)
        dsb[:, k, 0:w] = dpad[:, 0, k * CCH : k * CCH + w]
        dsb[:, 16 + k, 0:w] = dpad[:, 1, k * CCH : k * CCH + w]

    base = {"wt2": wt2, "bias2": b2}
    return [
        {
            "x2e": np.ascontiguousarray(x2e[i]),
            "dsb": np.ascontiguousarray(dsb[i]),
            **base,
        }
        for i in range(n)
    ]


def kernel(x, depth, weights, bias):
    nc = _get_program()
    if not nc.is_finalized():
        nc.finalize()
    in_maps = make_in_maps(x, depth, weights, bias)
    res = run_bass_kernel_spmd(nc, in_maps, list(range(NCORES)))
    out = np.stack([np.asarray(res.results[i]["out"]) for i in range(NCORES)])
    return out.astype(np.float32)
